# revision 18
# baseline (speedup 1.0000x reference)
"""Multi-head attention (B=4, S=2048, D=1024, H=16) on 8 NeuronCores.

Sharding: core (b, hg) with b = cid//2, hg = cid%2 computes the partial
output contribution of head-group hg (8 heads) of batch b:
    part = softmax((x_q Wq_hg^T + bq_hg)(x_k Wq_hg^T + bq_hg)^T / 8) (x_v ...) Wo[:, hg]^T
Host sums the two partials per batch and adds bo.

Kernel internals (per core):
  phase 1: DMA-transpose inputs to xT [D, S]; in-proj matmuls (bf16)
           producing qpT/kpT [512, 2048] (dims on partitions) and vp
           natural [2048, 512] with a ones column interleaved per head
           (vp_aug [2048, 8*65]) so the PV matmul also emits the softmax
           denominator as an extra output row.  DMA work is spread over
           the gpsimd (loads), sync+scalar (transposes) and vector
           (weights) queues so no single queue serializes the phase.
  phase 2: per head pair and q-block of 512, software-pipelined over kc:
           scoresT [k,q] matmuls (row-tiled head pairs), exp split
           ACT/DVE (Schraudolph bitcast on DVE), PV matmuls accumulate
           ctxT_aug [65, 2*512].  Double-buffered score/context PSUM (8
           banks total) keeps the PE stream dense so HAM stays warm.
           Normalization: denominator row -> spread across partitions by
           DMA -> cheap reciprocal -> broadcast back -> multiply.
  phase 3: out-proj (bf16) from concT [512, 2048], PSUM->SBUF->DRAM.
"""

import math

import ml_dtypes
import numpy as np

import concourse.bass as bass
from concourse import bacc
import concourse.mybir as mybir
import concourse.tile as tile

f32 = mybir.dt.float32
bf16 = mybir.dt.bfloat16
AF = mybir.ActivationFunctionType
i16 = mybir.dt.int16
# Schraudolph exp for bf16 bit pattern: bf16_bits = round(2^7*(s*0.125/ln2 + 127 - c))
SCHRAUD_A = 128.0 * 0.125 / math.log(2.0)
SCHRAUD_B = 128.0 * (127.0 - 0.0450466) + 0.5

P = 128
S = 2048           # sequence length
D = 1024           # model dim
DH = 512           # head-group dim (8 heads x 64)
HD = 64            # head dim
NH = 8             # heads per core
SC = S // P        # 16 seq chunks
KC = D // P        # 8 contraction chunks (model dim)
MC = DH // P       # 4 out-dim chunks
QG = 512           # q-block size in phase 2


def _pbcast(ap_, n):
    """AP reading ap_'s single partition replicated across n partitions."""
    return bass.AP(
        tensor=ap_.tensor, offset=ap_.offset, ap=[[0, n]] + [list(d) for d in ap_.ap[1:]]
    )


# exp engine pattern per (kc % 8, head-in-pair): 9 ACT / 7 DVE per 16 tiles
_EXP_ENG = {
    0: ("A", "D"), 1: ("A", "A"), 2: ("D", "A"), 3: ("A", "D"),
    4: ("D", "A"), 5: ("A", "D"), 6: ("D", "A"), 7: ("A", "D"),
}


def build_kernel():
    nc = bacc.Bacc(None, target_bir_lowering=False)
    # inputs arrive pre-transposed and pre-cast to bf16 from the host
    xqt = nc.dram_tensor("xqt", [D, S], bf16, kind="ExternalInput")
    xkt = nc.dram_tensor("xkt", [D, S], bf16, kind="ExternalInput")
    xvt = nc.dram_tensor("xvt", [D, S], bf16, kind="ExternalInput")
    wqt = nc.dram_tensor("wqt", [D, DH], bf16, kind="ExternalInput")  # Wq_hg.T
    bq = nc.dram_tensor("bq", [DH], f32, kind="ExternalInput")
    wot = nc.dram_tensor("wot", [DH, D], bf16, kind="ExternalInput")  # Wo[:, hg].T
    onesc = nc.dram_tensor("onesc", [SC, NH], bf16, kind="ExternalInput")
    out = nc.dram_tensor("out", [S, D], f32, kind="ExternalOutput")

    with tile.TileContext(nc) as tc:
        with tc.tile_pool(name="singles", bufs=1) as singles:
            # ---- constants / weights ----
            WQT = singles.tile([P, KC, DH], bf16)
            nc.scalar.dma_start(WQT, wqt[:].rearrange("(kc p) m -> p kc m", p=P))
            BQT = singles.tile([P, MC], f32)
            nc.scalar.dma_start(BQT, bq[:].rearrange("(mc p) -> p mc", p=P))
            BQB = singles.tile([P, DH], f32)
            nc.gpsimd.dma_start(BQB, bq[:].partition_broadcast(P))
            ones_sb = singles.tile([P, SC * NH], bf16)
            nc.gpsimd.dma_start(
                ones_sb.rearrange("p (sc h) -> p sc h", h=NH),
                bass.AP(
                    tensor=onesc[:].tensor, offset=0,
                    ap=[[0, P], [NH, SC], [1, NH]],
                ),
            )
            WOT = singles.tile([P, MC, D], bf16)

            # ---- persistent activations ----
            QPT = singles.tile([P, MC, S], bf16)    # qpT: [dim, seq]
            KPT = singles.tile([P, MC, S], bf16)
            CONCT = singles.tile([P, MC, S], bf16)
            VPA = singles.tile([P, SC, NH * (HD + 1)], bf16)  # vp + ones cols
            vones = (
                VPA[:, :, :]
                .rearrange("p sc (h c) -> p sc h c", h=NH)[:, :, :, HD:HD + 1]
            )
            nc.vector.tensor_copy(
                vones,
                ones_sb.rearrange("p (sc h) -> p sc h", h=NH).unsqueeze(3),
            )

            # =========== phase 1: load pre-transposed inputs + projections ===========
            with (
                tc.tile_pool(name="xt", bufs=3) as xt_pool,
                tc.tile_pool(name="pps", bufs=6, space="PSUM") as ppool,
            ):
                for g in range(4):            # groups of 512 seq positions
                    gsl = slice(g * 512, (g + 1) * 512)
                    for t, xin in enumerate((xqt, xkt, xvt)):
                        xt = xt_pool.tile([P, KC, 512], bf16, tag="xt")
                        nc.sync.dma_start(
                            xt,
                            xin[:, gsl].rearrange("(kc p) s -> p kc s", p=P),
                        )
                        if t < 2:
                            dst = QPT if t == 0 else KPT
                            for mc in range(MC):
                                ps = ppool.tile([P, 512], f32, tag="pp")
                                for kc in range(KC):
                                    nc.tensor.matmul(
                                        ps,
                                        WQT[:, kc, mc * P:(mc + 1) * P],
                                        xt[:, kc, :],
                                        start=(kc == 0),
                                        stop=(kc == KC - 1),
                                    )
                                nc.scalar.activation(
                                    dst[:, mc, g * 512:(g + 1) * 512],
                                    ps,
                                    AF.Identity,
                                    bias=BQT[:, mc:mc + 1],
                                    scale=1.0,
                                )
                        else:
                            for m in range(4):
                                sc = g * 4 + m
                                ps = ppool.tile([P, 512], f32, tag="pp")
                                for kc in range(KC):
                                    nc.tensor.matmul(
                                        ps,
                                        xt[:, kc, m * P:(m + 1) * P],
                                        WQT[:, kc, :],
                                        start=(kc == 0),
                                        stop=(kc == KC - 1),
                                    )
                                nc.vector.tensor_add(
                                    VPA[:, sc, :]
                                    .rearrange("p (h c) -> p h c", h=NH)[:, :, 0:HD],
                                    ps.rearrange("p (h c) -> p h c", h=NH),
                                    BQB.rearrange("p (h c) -> p h c", h=NH),
                                )

            # WOT only needed in phase 3 — load it during phase 2
            nc.scalar.dma_start(WOT, wot[:].rearrange("(mc p) n -> p mc n", p=P))

            # =========== phase 2: attention ===========
            with (
                tc.tile_pool(name="att", bufs=2) as at_pool,
                tc.tile_pool(name="dsb", bufs=2) as ds_pool,
                tc.tile_pool(name="rcp", bufs=2) as rc_pool,
                tc.tile_pool(name="tmu", bufs=2) as tm_pool,
                tc.tile_pool(name="rcd", bufs=2, space="DRAM") as rd_pool,
                tc.tile_pool(name="sps", bufs=2, space="PSUM") as sc_ps,
                tc.tile_pool(name="cps", bufs=2, space="PSUM") as ctx_ps,
            ):
                act_credit = 0.0
                ACT_RATIO = 0.568
                for hp in range(4):           # head pairs
                    for qg in range(S // QG):
                        qsl = slice(qg * QG, (qg + 1) * QG)
                        cps = ctx_ps.tile([HD + 1, 2 * QG], f32, tag="cp")
                        atts = {}
                        # software pipeline over kc-pairs: scores/exp at j,
                        # PV at j-1.  Each exp op covers two kc chunks.
                        for j in range(SC // 2 + 1):
                            if j < SC // 2:
                                spss = {}
                                for hi, po in ((0, 0), (1, HD)):
                                    spss[hi] = sc_ps.tile(
                                        [P, 2 * QG], f32, tag="sp",
                                        name=f"sp{hi}_{j}",
                                    )
                                for par in (0, 1):
                                    kc = 2 * j + par
                                    for hi, po in ((0, 0), (1, HD)):
                                        nc.tensor.matmul(
                                            spss[hi][:, par * QG:(par + 1) * QG],
                                            KPT[po:po + HD, hp, kc * P:(kc + 1) * P],
                                            QPT[po:po + HD, hp, qsl],
                                            start=True,
                                            stop=True,
                                        )
                                for hi, po in ((0, 0), (1, HD)):
                                    att = at_pool.tile(
                                        [P, 2 * QG], bf16, tag=f"a{po}"
                                    )
                                    act_credit += ACT_RATIO
                                    if act_credit >= 1.0:
                                        act_credit -= 1.0
                                        nc.scalar.activation(
                                            att, spss[hi], AF.Exp, scale=0.125
                                        )
                                    else:
                                        nc.vector.tensor_scalar(
                                            att.bitcast(i16), spss[hi],
                                            SCHRAUD_A, SCHRAUD_B,
                                            op0=mybir.AluOpType.mult,
                                            op1=mybir.AluOpType.add,
                                        )
                                    atts[(j, hi)] = att
                            if j >= 1:
                                pj = j - 1
                                for hi, po in ((0, 0), (1, HD)):
                                    h = 2 * hp + hi
                                    att = atts.pop((pj, hi))
                                    for par in (0, 1):
                                        kc = 2 * pj + par
                                        nc.tensor.matmul(
                                            cps[:, hi * QG:(hi + 1) * QG],
                                            VPA[:, kc, h * (HD + 1):(h + 1) * (HD + 1)],
                                            att[:, par * QG:(par + 1) * QG],
                                            start=(kc == 0),
                                            stop=(kc == SC - 1),
                                        )
                        # ---- normalize tail ----
                        dsb = ds_pool.tile([1, 2 * QG], f32, tag="dsb")
                        nc.scalar.copy(dsb, cps[HD:HD + 1, :])
                        d1 = rd_pool.tile([1, 2 * QG], f32, tag="d1")
                        nc.sync.dma_start(d1, dsb)
                        dsp = ds_pool.tile([P, (2 * QG) // P], f32, tag="dsp")
                        nc.gpsimd.dma_start(
                            dsp, d1[:].rearrange("o (p f) -> (o p) f", p=P)
                        )
                        dsr = ds_pool.tile([P, (2 * QG) // P], f32, tag="dsr")
                        nc.vector.reciprocal(dsr, dsp)
                        d2 = rd_pool.tile([1, 2 * QG], f32, tag="d2")
                        nc.sync.dma_start(
                            d2[:].rearrange("o (p f) -> (o p) f", p=P), dsr
                        )
                        rep = rc_pool.tile([HD, 2 * QG], f32, tag="rep")
                        nc.gpsimd.dma_start(
                            rep,
                            d2[:].rearrange("o f -> (o f)").partition_broadcast(HD),
                        )
                        nc.vector.tensor_mul(
                            CONCT[0:HD, hp, qsl], cps[0:HD, 0:QG], rep[:, 0:QG]
                        )
                        tmp = tm_pool.tile([HD, QG], bf16, tag="tm")
                        nc.vector.tensor_mul(
                            tmp, cps[0:HD, QG:2 * QG], rep[:, QG:2 * QG]
                        )
                        nc.sync.dma_start(CONCT[HD:P, hp, qsl], tmp)

            # =========== phase 3: output projection ===========
            with (
                tc.tile_pool(name="ops", bufs=4, space="PSUM") as out_ps,
                tc.tile_pool(name="osb", bufs=4) as out_sb,
            ):
                oq = 0
                for sc in range(SC):
                    for n in range(D // 512):
                        ps = out_ps.tile([P, 512], f32, tag="op")
                        for mc in range(MC):
                            nc.tensor.matmul(
                                ps,
                                CONCT[:, mc, sc * P:(sc + 1) * P],
                                WOT[:, mc, n * 512:(n + 1) * 512],
                                start=(mc == 0),
                                stop=(mc == MC - 1),
                            )
                        osb = out_sb.tile([P, 512], f32, tag="ob")
                        nc.scalar.copy(osb, ps)
                        oq += 1
                        nc.sync.dma_start(
                            out[sc * P:(sc + 1) * P, n * 512:(n + 1) * 512], osb
                        )
    nc.finalize()
    return nc


_NC = None


def _get_nc():
    global _NC
    if _NC is None:
        _NC = build_kernel()
    return _NC


def kernel(q, k, v, Wq, bq, Wo, bo, _trace=False):
    from concourse.bass_utils import run_bass_kernel_spmd

    q = np.asarray(q, dtype=np.float32)
    k = np.asarray(k, dtype=np.float32)
    v = np.asarray(v, dtype=np.float32)
    Wq = np.asarray(Wq, dtype=np.float32)
    bq = np.asarray(bq, dtype=np.float32)
    Wo = np.asarray(Wo, dtype=np.float32)
    bo = np.asarray(bo, dtype=np.float32)

    nc = _get_nc()
    B = q.shape[0]
    bf = ml_dtypes.bfloat16
    qT = [np.ascontiguousarray(q[b].T.astype(bf)) for b in range(B)]
    kT = [np.ascontiguousarray(k[b].T.astype(bf)) for b in range(B)]
    vT = [np.ascontiguousarray(v[b].T.astype(bf)) for b in range(B)]
    in_maps = []
    for cid in range(8):
        b, hg = cid // 2, cid % 2
        sl = slice(hg * DH, (hg + 1) * DH)
        in_maps.append({
            "xqt": qT[b],
            "xkt": kT[b],
            "xvt": vT[b],
            "wqt": np.ascontiguousarray(Wq[sl, :].T.astype(bf)),
            "bq": np.ascontiguousarray(bq[sl]),
            "wot": np.ascontiguousarray(Wo[:, sl].T.astype(bf)),
            "onesc": np.ones((SC, NH), dtype=bf),
        })
    kw = {}
    if _trace:
        import os
        import shutil

        td = "/tmp/ntff_out"
        shutil.rmtree(td, ignore_errors=True)
        os.makedirs(td, exist_ok=True)
        kw["tmpdir"] = td
    res = run_bass_kernel_spmd(
        nc, in_maps, core_ids=list(range(8)), trace=_trace, **kw
    )
    parts = [r["out"] for r in res.results]
    outv = np.stack([parts[2 * b] + parts[2 * b + 1] for b in range(B)])
    outv = outv + bo[None, None, :]
    if _trace:
        kernel.last_result = res
    return outv[None].astype(np.float32)


# revision 19
# speedup vs baseline: 1.0446x; 1.0446x over previous
"""Multi-head attention (B=4, S=2048, D=1024, H=16) on 8 NeuronCores.

Sharding: core (b, hg) with b = cid//2, hg = cid%2 computes the partial
output contribution of head-group hg (8 heads) of batch b:
    part = softmax((x_q Wq_hg^T + bq_hg)(x_k Wq_hg^T + bq_hg)^T / 8) (x_v ...) Wo[:, hg]^T
Host sums the two partials per batch and adds bo.

Kernel internals (per core):
  phase 1: DMA-transpose inputs to xT [D, S]; in-proj matmuls (bf16)
           producing qpT/kpT [512, 2048] (dims on partitions) and vp
           natural [2048, 512] with a ones column interleaved per head
           (vp_aug [2048, 8*65]) so the PV matmul also emits the softmax
           denominator as an extra output row.  DMA work is spread over
           the gpsimd (loads), sync+scalar (transposes) and vector
           (weights) queues so no single queue serializes the phase.
  phase 2: per head pair and q-block of 512, software-pipelined over kc:
           scoresT [k,q] matmuls (row-tiled head pairs), exp split
           ACT/DVE (Schraudolph bitcast on DVE), PV matmuls accumulate
           ctxT_aug [65, 2*512].  Double-buffered score/context PSUM (8
           banks total) keeps the PE stream dense so HAM stays warm.
           Normalization: denominator row -> spread across partitions by
           DMA -> cheap reciprocal -> broadcast back -> multiply.
  phase 3: out-proj (bf16) from concT [512, 2048], PSUM->SBUF->DRAM.
"""

import math

import ml_dtypes
import numpy as np

import concourse.bass as bass
from concourse import bacc
import concourse.mybir as mybir
import concourse.tile as tile

f32 = mybir.dt.float32
bf16 = mybir.dt.bfloat16
AF = mybir.ActivationFunctionType
i16 = mybir.dt.int16
# Schraudolph exp for bf16 bit pattern: bf16_bits = round(2^7*(s*0.125/ln2 + 127 - c))
SCHRAUD_A = 128.0 * 0.125 / math.log(2.0)
SCHRAUD_B = 128.0 * (127.0 - 0.0450466) + 0.5

P = 128
S = 2048           # sequence length
D = 1024           # model dim
DH = 512           # head-group dim (8 heads x 64)
HD = 64            # head dim
NH = 8             # heads per core
SC = S // P        # 16 seq chunks
KC = D // P        # 8 contraction chunks (model dim)
MC = DH // P       # 4 out-dim chunks
QG = 512           # q-block size in phase 2


def _pbcast(ap_, n):
    """AP reading ap_'s single partition replicated across n partitions."""
    return bass.AP(
        tensor=ap_.tensor, offset=ap_.offset, ap=[[0, n]] + [list(d) for d in ap_.ap[1:]]
    )


# exp engine pattern per (kc % 8, head-in-pair): 9 ACT / 7 DVE per 16 tiles
_EXP_ENG = {
    0: ("A", "D"), 1: ("A", "A"), 2: ("D", "A"), 3: ("A", "D"),
    4: ("D", "A"), 5: ("A", "D"), 6: ("D", "A"), 7: ("A", "D"),
}


def build_kernel():
    nc = bacc.Bacc(None, target_bir_lowering=False)
    # inputs arrive pre-transposed and pre-cast to bf16 from the host
    xqt = nc.dram_tensor("xqt", [D, S], bf16, kind="ExternalInput")
    xkt = nc.dram_tensor("xkt", [D, S], bf16, kind="ExternalInput")
    xvt = nc.dram_tensor("xvt", [D, S], bf16, kind="ExternalInput")
    wqt = nc.dram_tensor("wqt", [D, DH], bf16, kind="ExternalInput")  # Wq_hg.T
    bq = nc.dram_tensor("bq", [DH], f32, kind="ExternalInput")
    wot = nc.dram_tensor("wot", [DH, D], bf16, kind="ExternalInput")  # Wo[:, hg].T
    onesc = nc.dram_tensor("onesc", [SC, NH], bf16, kind="ExternalInput")
    out = nc.dram_tensor("out", [S, D], f32, kind="ExternalOutput")

    with tile.TileContext(nc) as tc:
        with tc.tile_pool(name="singles", bufs=1) as singles:
            # ---- constants / weights ----
            WQT = singles.tile([P, KC, DH], bf16)
            nc.scalar.dma_start(WQT, wqt[:].rearrange("(kc p) m -> p kc m", p=P))
            BQT = singles.tile([P, MC], f32)
            nc.scalar.dma_start(BQT, bq[:].rearrange("(mc p) -> p mc", p=P))
            BQB = singles.tile([P, DH], f32)
            nc.gpsimd.dma_start(BQB, bq[:].partition_broadcast(P))
            ones_sb = singles.tile([P, SC * NH], bf16)
            nc.gpsimd.dma_start(
                ones_sb.rearrange("p (sc h) -> p sc h", h=NH),
                bass.AP(
                    tensor=onesc[:].tensor, offset=0,
                    ap=[[0, P], [NH, SC], [1, NH]],
                ),
            )
            WOT = singles.tile([P, MC, D], bf16)

            # ---- persistent activations ----
            QPT = singles.tile([P, MC, S], bf16)    # qpT: [dim, seq]
            KPT = singles.tile([P, MC, S], bf16)
            CONCT = singles.tile([P, MC, S], bf16)
            VPA = singles.tile([P, SC, NH * (HD + 1)], bf16)  # vp + ones cols
            vones = (
                VPA[:, :, :]
                .rearrange("p sc (h c) -> p sc h c", h=NH)[:, :, :, HD:HD + 1]
            )
            nc.vector.tensor_copy(
                vones,
                ones_sb.rearrange("p (sc h) -> p sc h", h=NH).unsqueeze(3),
            )

            # =========== phase 1: load pre-transposed inputs + projections ===========
            with (
                tc.tile_pool(name="xt", bufs=3) as xt_pool,
                tc.tile_pool(name="pps", bufs=6, space="PSUM") as ppool,
            ):
                for g in range(4):            # groups of 512 seq positions
                    gsl = slice(g * 512, (g + 1) * 512)
                    for t, xin in enumerate((xqt, xkt, xvt)):
                        xt = xt_pool.tile([P, KC, 512], bf16, tag="xt")
                        nc.sync.dma_start(
                            xt,
                            xin[:, gsl].rearrange("(kc p) s -> p kc s", p=P),
                        )
                        if t < 2:
                            dst = QPT if t == 0 else KPT
                            for mc in range(MC):
                                ps = ppool.tile([P, 512], f32, tag="pp")
                                for kc in range(KC):
                                    nc.tensor.matmul(
                                        ps,
                                        WQT[:, kc, mc * P:(mc + 1) * P],
                                        xt[:, kc, :],
                                        start=(kc == 0),
                                        stop=(kc == KC - 1),
                                    )
                                nc.scalar.activation(
                                    dst[:, mc, g * 512:(g + 1) * 512],
                                    ps,
                                    AF.Identity,
                                    bias=BQT[:, mc:mc + 1],
                                    scale=1.0,
                                )
                        else:
                            for m in range(4):
                                sc = g * 4 + m
                                ps = ppool.tile([P, 512], f32, tag="pp")
                                for kc in range(KC):
                                    nc.tensor.matmul(
                                        ps,
                                        xt[:, kc, m * P:(m + 1) * P],
                                        WQT[:, kc, :],
                                        start=(kc == 0),
                                        stop=(kc == KC - 1),
                                    )
                                nc.vector.tensor_add(
                                    VPA[:, sc, :]
                                    .rearrange("p (h c) -> p h c", h=NH)[:, :, 0:HD],
                                    ps.rearrange("p (h c) -> p h c", h=NH),
                                    BQB.rearrange("p (h c) -> p h c", h=NH),
                                )

            # WOT only needed in phase 3 — load it during phase 2
            nc.scalar.dma_start(WOT, wot[:].rearrange("(mc p) n -> p mc n", p=P))

            # =========== phase 2: attention ===========
            with (
                tc.tile_pool(name="att", bufs=2) as at_pool,
                tc.tile_pool(name="dsb", bufs=2) as ds_pool,
                tc.tile_pool(name="rcp", bufs=2) as rc_pool,
                tc.tile_pool(name="tmu", bufs=2) as tm_pool,
                tc.tile_pool(name="rcd", bufs=2, space="DRAM") as rd_pool,
                tc.tile_pool(name="sps", bufs=2, space="PSUM") as sc_ps,
                tc.tile_pool(name="cps", bufs=2, space="PSUM") as ctx_ps,
            ):
                # deferred-tail state: block n's normalize lane-ops run
                # during block n+1 so DMA latency never stalls the queues
                pend = []   # list of dicts with per-block tail state

                def tail_stage1(st):
                    # reciprocal + broadcast DMA (inputs long since landed)
                    dsr = ds_pool.tile(
                        [P, (2 * QG) // P], f32, tag="dsr", name="dsr"
                    )
                    nc.vector.reciprocal(dsr, st["dsp"])
                    d2 = rd_pool.tile([1, 2 * QG], f32, tag="d2", name="d2")
                    nc.sync.dma_start(
                        d2[:].rearrange("o (p f) -> (o p) f", p=P), dsr
                    )
                    rep = rc_pool.tile([HD, 2 * QG], f32, tag="rep", name="rep")
                    nc.gpsimd.dma_start(
                        rep,
                        d2[:].rearrange("o f -> (o f)").partition_broadcast(HD),
                    )
                    st["rep"] = rep

                def tail_stage2(st):
                    hp_, qsl_, cps_, rep_ = st["hp"], st["qsl"], st["cps"], st["rep"]
                    nc.vector.tensor_mul(
                        CONCT[0:HD, hp_, qsl_], cps_[0:HD, 0:QG], rep_[:, 0:QG]
                    )
                    tmp = tm_pool.tile([HD, QG], bf16, tag="tm", name="tmp")
                    nc.vector.tensor_mul(
                        tmp, cps_[0:HD, QG:2 * QG], rep_[:, QG:2 * QG]
                    )
                    nc.sync.dma_start(CONCT[HD:P, hp_, qsl_], tmp)

                for hp in range(4):           # head pairs
                    for qg in range(S // QG):
                        qsl = slice(qg * QG, (qg + 1) * QG)
                        cps = ctx_ps.tile([HD + 1, 2 * QG], f32, tag="cp")
                        if pend:
                            tail_stage1(pend[-1])
                        atts = {}
                        # software pipeline: scores/exp at kc, PV at kc-1
                        for kc in range(SC + 1):
                            if kc < SC:
                                for hi, po in ((0, 0), (1, HD)):
                                    sps = sc_ps.tile([P, QG], f32, tag=f"s{po}")
                                    nc.tensor.matmul(
                                        sps,
                                        KPT[po:po + HD, hp, kc * P:(kc + 1) * P],
                                        QPT[po:po + HD, hp, qsl],
                                        start=True,
                                        stop=True,
                                    )
                                    att = at_pool.tile([P, QG], bf16, tag=f"a{po}")
                                    if _EXP_ENG[kc % 8][hi] == "A":
                                        nc.scalar.activation(
                                            att, sps, AF.Exp, scale=0.125
                                        )
                                    else:
                                        nc.vector.tensor_scalar(
                                            att.bitcast(i16), sps,
                                            SCHRAUD_A, SCHRAUD_B,
                                            op0=mybir.AluOpType.mult,
                                            op1=mybir.AluOpType.add,
                                        )
                                    atts[(kc, hi)] = att
                            if kc == SC // 2 and pend:
                                tail_stage2(pend.pop())
                            if kc >= 1:
                                pk = kc - 1
                                for hi, po in ((0, 0), (1, HD)):
                                    h = 2 * hp + hi
                                    nc.tensor.matmul(
                                        cps[:, hi * QG:(hi + 1) * QG],
                                        VPA[:, pk, h * (HD + 1):(h + 1) * (HD + 1)],
                                        atts.pop((pk, hi)),
                                        start=(pk == 0),
                                        stop=(pk == SC - 1),
                                    )
                        # ---- kick off this block's denominator DMA chain ----
                        dsb = ds_pool.tile([1, 2 * QG], f32, tag="dsb")
                        nc.scalar.copy(dsb, cps[HD:HD + 1, :])
                        d1 = rd_pool.tile([1, 2 * QG], f32, tag="d1")
                        nc.sync.dma_start(d1, dsb)
                        dsp = ds_pool.tile([P, (2 * QG) // P], f32, tag="dsp")
                        nc.gpsimd.dma_start(
                            dsp, d1[:].rearrange("o (p f) -> (o p) f", p=P)
                        )
                        pend.append(
                            {"hp": hp, "qsl": qsl, "cps": cps, "dsp": dsp}
                        )
                # drain the last block's tail
                st = pend.pop()
                tail_stage1(st)
                tail_stage2(st)

            # =========== phase 3: output projection ===========
            with (
                tc.tile_pool(name="ops", bufs=4, space="PSUM") as out_ps,
                tc.tile_pool(name="osb", bufs=4) as out_sb,
            ):
                oq = 0
                for sc in range(SC):
                    for n in range(D // 512):
                        ps = out_ps.tile([P, 512], f32, tag="op")
                        for mc in range(MC):
                            nc.tensor.matmul(
                                ps,
                                CONCT[:, mc, sc * P:(sc + 1) * P],
                                WOT[:, mc, n * 512:(n + 1) * 512],
                                start=(mc == 0),
                                stop=(mc == MC - 1),
                            )
                        osb = out_sb.tile([P, 512], f32, tag="ob")
                        nc.scalar.copy(osb, ps)
                        oq += 1
                        nc.sync.dma_start(
                            out[sc * P:(sc + 1) * P, n * 512:(n + 1) * 512], osb
                        )
    nc.finalize()
    return nc


_NC = None


def _get_nc():
    global _NC
    if _NC is None:
        _NC = build_kernel()
    return _NC


def kernel(q, k, v, Wq, bq, Wo, bo, _trace=False):
    from concourse.bass_utils import run_bass_kernel_spmd

    q = np.asarray(q, dtype=np.float32)
    k = np.asarray(k, dtype=np.float32)
    v = np.asarray(v, dtype=np.float32)
    Wq = np.asarray(Wq, dtype=np.float32)
    bq = np.asarray(bq, dtype=np.float32)
    Wo = np.asarray(Wo, dtype=np.float32)
    bo = np.asarray(bo, dtype=np.float32)

    nc = _get_nc()
    B = q.shape[0]
    bf = ml_dtypes.bfloat16
    qT = [np.ascontiguousarray(q[b].T.astype(bf)) for b in range(B)]
    kT = [np.ascontiguousarray(k[b].T.astype(bf)) for b in range(B)]
    vT = [np.ascontiguousarray(v[b].T.astype(bf)) for b in range(B)]
    in_maps = []
    for cid in range(8):
        b, hg = cid // 2, cid % 2
        sl = slice(hg * DH, (hg + 1) * DH)
        in_maps.append({
            "xqt": qT[b],
            "xkt": kT[b],
            "xvt": vT[b],
            "wqt": np.ascontiguousarray(Wq[sl, :].T.astype(bf)),
            "bq": np.ascontiguousarray(bq[sl]),
            "wot": np.ascontiguousarray(Wo[:, sl].T.astype(bf)),
            "onesc": np.ones((SC, NH), dtype=bf),
        })
    kw = {}
    if _trace:
        import os
        import shutil

        td = "/tmp/ntff_out"
        shutil.rmtree(td, ignore_errors=True)
        os.makedirs(td, exist_ok=True)
        kw["tmpdir"] = td
    res = run_bass_kernel_spmd(
        nc, in_maps, core_ids=list(range(8)), trace=_trace, **kw
    )
    parts = [r["out"] for r in res.results]
    outv = np.stack([parts[2 * b] + parts[2 * b + 1] for b in range(B)])
    outv = outv + bo[None, None, :]
    if _trace:
        kernel.last_result = res
    return outv[None].astype(np.float32)
